# revision 31
# baseline (speedup 1.0000x reference)
"""GQA attention block (B=2, T=2048, C=2048, H=32, Hkv=8, D=64, RoPE, causal)
on 8 TRN2 NeuronCores.

Sharding: core = b*4 + g  (b = batch 0..1, g = head-group 0..3).
Each core computes 8 Q heads / 2 KV heads of one batch element:
  QKV projections -> RoPE -> causal softmax(QK^T/sqrt(D)) V -> partial
  output projection against its 512 columns of Wc.  Host sums the 4
  head-group partials per batch (partials are written in bf16).

Pipeline structure (per core): one fused loop over the 4 sequence blocks.
Iteration tb projects Q/K/V for t-block tb, then runs attention for
q-block tb (which only needs K/V up to block tb), then the output
projection for those rows.  Projection matmuls (PE-heavy) overlap the
previous block's attention (ScalarE-exp-heavy) in the Tile schedule.

Attention computes S^T = K Q^T tiles (k on partitions) so the exp'd
tiles feed the PV matmul with no transposes; a ones-column appended to V
accumulates the softmax denominator in the same matmul; causal masking
skips fully-masked tiles, narrows diagonal-crossing streams, and applies
a 128x128 triangle mask (on GpSimd) to the diagonal block.

Every matmul runs in the full 128x128 PE mode to avoid tile-mode-switch
drains: the per-head K tiles are stored zero-padded to 128 contraction
rows (the other head's rows are 0).  All DRAM inputs are host-packed so
each SBUF tile is a contiguous per-partition read, and output partials
are written in bf16.

Matmul operands are bf16 (KERNEL_MM_DTYPE=f32r selects float32r:
slower, lower error); PSUM accumulation is always fp32.
"""

import os

import ml_dtypes
import numpy as np

import concourse.bacc as bacc
import concourse.mybir as mybir
from concourse.tile import TileContext
from concourse.bass_utils import run_bass_kernel_spmd

B, T, C = 2, 2048, 2048
H, HKV, D = 32, 8, 64
ROPE_THETA = 10000.0

P = 128
NCT = C // P          # 16 contraction subtiles
TB = 512              # t-block width
NTB = T // TB         # 4
QB = 512              # q-block width in attention
KT = T // P           # 16 k-tiles
QH = H // 4           # 8 local q heads per core
LOCAL_HEADS = [0, 4, 1, 5, 2, 6, 3, 7]  # pair (p, p+4) shares a 128-row tile

F32 = mybir.dt.float32
F32R = mybir.dt.float32r
BF16 = mybir.dt.bfloat16

MM_MODE = os.environ.get("KERNEL_MM_DTYPE", "bf16")
MMDT = BF16 if MM_MODE == "bf16" else F32R
NPDT = ml_dtypes.bfloat16 if MM_MODE == "bf16" else np.float32

EXP_SCALE = float(1.0 / np.sqrt(D))


def build_bass():
    nc = bacc.Bacc("TRN2", target_bir_lowering=False, debug=False, num_devices=8)

    # All inputs are pre-packed on the host so every SBUF tile is a
    # contiguous per-partition DRAM read (128 x >=4KB DMA descriptors).
    xP = nc.dram_tensor("xP", [P, NTB, NCT, TB], MMDT, kind="ExternalInput")
    wqP = nc.dram_tensor("wqP", [P, 4, NCT, P], MMDT, kind="ExternalInput")
    wkP = nc.dram_tensor("wkP", [P, NCT, 2 * D], MMDT, kind="ExternalInput")
    wvP = nc.dram_tensor("wvP", [P, NCT, 2 * D], MMDT, kind="ExternalInput")
    wcP = nc.dram_tensor("wcP", [P, 4, 4, 512], MMDT, kind="ExternalInput")
    cosT = nc.dram_tensor("cosT", [P, T], MMDT, kind="ExternalInput")
    sinT = nc.dram_tensor("sinT", [P, T], MMDT, kind="ExternalInput")
    tri = nc.dram_tensor("tri", [P, 2, P], MMDT, kind="ExternalInput")
    ident = nc.dram_tensor("ident", [P, P], MMDT, kind="ExternalInput")
    out = nc.dram_tensor("out", [T, C], MMDT, kind="ExternalOutput")

    with TileContext(nc) as tc:
        with (
            tc.tile_pool(name="persist", bufs=1) as persist,
            tc.tile_pool(name="small", bufs=6) as small,
            tc.tile_pool(name="xs", bufs=8) as xs,
            tc.tile_pool(name="rot", bufs=3) as rotp,
            tc.tile_pool(name="vt", bufs=2) as vtp,
            tc.tile_pool(name="pt", bufs=8) as ptp,
            tc.tile_pool(name="ostage", bufs=2) as ostage,
            tc.tile_pool(name="psMM", bufs=2, space="PSUM") as psMM,
            tc.tile_pool(name="psST", bufs=2, space="PSUM") as psST,
            tc.tile_pool(name="psPV", bufs=2, space="PSUM") as psPV,
        ):
            # ---- persistent SBUF tensors ------------------------------
            q_sb = persist.tile([P, 4, T], MMDT)          # Q^T (rope'd)
            # per-head K^T, zero-padded to 128 contraction rows so the
            # QK matmul stays in 128x128 PE mode (no tile-mode switch)
            k_sb = [
                persist.tile([P, T], MMDT, tag=f"k{h}", name=f"k{h}")
                for h in range(2)
            ]
            v_sb = persist.tile([P, KT, 2, D + 1], MMDT)  # V + ones col
            y_sb = persist.tile([P, 4, T], MMDT)          # attn out^T
            tri_sb = persist.tile([P, 2, P], MMDT)
            id_sb = persist.tile([P, P], MMDT)
            cos_sb = persist.tile([P, T], MMDT)
            sin_sb = persist.tile([P, T], MMDT)
            wk_sb = persist.tile([P, NCT, 2 * D], MMDT, tag="wk")
            wv_sb = persist.tile([P, NCT, 2 * D], MMDT, tag="wv")
            wq_sb = persist.tile([P, 4, NCT, P], MMDT, tag="wq")
            wc_sb = persist.tile([P, 4, 4, 512], MMDT, tag="wc")

            nc.sync.dma_start(wk_sb[:], wkP[:])
            # zero the dead half of each padded K tile (once)
            nc.vector.memset(k_sb[0][D:P, :], 0.0)
            nc.vector.memset(k_sb[1][0:D, :], 0.0)
            nc.vector.memset(v_sb[:, :, :, D], 1.0)  # softmax-denominator col

            def rope_q(dst, psum, tb):
                # dst/psum: [128, TB]; q_rope = q*cos + rot(q)*sin'
                tmp = rotp.tile([P, TB], MMDT, tag="rp_t")
                rtmp = rotp.tile([P, TB], MMDT, tag="rp_r")
                nc.scalar.copy(tmp[:], psum[:])
                for olo, ilo in ((0, 32), (32, 0), (64, 96), (96, 64)):
                    nc.scalar.dma_start(
                        rtmp[olo : olo + 32, :], tmp[ilo : ilo + 32, :]
                    )
                ts = slice(tb * TB, (tb + 1) * TB)
                nc.vector.tensor_mul(dst, tmp[:], cos_sb[:, ts])
                nc.vector.tensor_mul(rtmp[:], rtmp[:], sin_sb[:, ts])
                nc.vector.tensor_add(dst, dst, rtmp[:])

            def rope_k(psum, tb):
                # like rope_q but writes each head's rows into its padded tile
                tmp = rotp.tile([P, TB], MMDT, tag="rp_t")
                rtmp = rotp.tile([P, TB], MMDT, tag="rp_r")
                nc.scalar.copy(tmp[:], psum[:])
                for olo, ilo in ((0, 32), (32, 0), (64, 96), (96, 64)):
                    nc.scalar.dma_start(
                        rtmp[olo : olo + 32, :], tmp[ilo : ilo + 32, :]
                    )
                ts = slice(tb * TB, (tb + 1) * TB)
                for hh in range(2):
                    rs = slice(hh * D, (hh + 1) * D)
                    dst = k_sb[hh][rs, ts]
                    nc.vector.tensor_mul(dst, tmp[rs, :], cos_sb[rs, ts])
                    nc.vector.tensor_mul(rtmp[rs, :], rtmp[rs, :], sin_sb[rs, ts])
                    nc.vector.tensor_add(dst, dst, rtmp[rs, :])

            def load_x(tb):
                xh = []
                for qtr in range(4):
                    xb = xs.tile(
                        [P, NCT // 4, TB], MMDT, tag="xb", name=f"xb{qtr}"
                    )
                    nc.sync.dma_start(
                        xb[:],
                        xP[:, tb, qtr * (NCT // 4) : (qtr + 1) * (NCT // 4), :],
                    )
                    xh.append(xb)
                return xh

            def proj_block(tb, xh):
                tsl = slice(tb * TB, (tb + 1) * TB)

                def xc(c):
                    return xh[c // (NCT // 4)][:, c % (NCT // 4), :]

                if tb == 0:
                    # weight loads ride the SWDGE queue so they stream in
                    # parallel with the x loads on the sync HWDGE ring;
                    # wq is split per m-tile so Q chains start early
                    for m in range(4):
                        nc.gpsimd.dma_start(wq_sb[:, m], wqP[:, m])
                    nc.sync.dma_start(cos_sb[:], cosT[:])
                    nc.sync.dma_start(sin_sb[:], sinT[:])
                    nc.gpsimd.dma_start(wv_sb[:], wvP[:])
                    nc.sync.dma_start(tri_sb[:], tri[:])
                    nc.sync.dma_start(id_sb[:], ident[:])
                    for cb in range(4):
                        nc.gpsimd.dma_start(wc_sb[:, cb], wcP[:, cb])

                # ---- K^T projection (one [128, TB] tile: 2 kv heads) --
                pk = psMM.tile([P, TB], F32, tag="mm512", name="pk")
                for c in range(NCT):
                    nc.tensor.matmul(
                        pk[:], wk_sb[:, c, :], xc(c),
                        start=(c == 0), stop=(c == NCT - 1),
                    )
                rope_k(pk, tb)

                # ---- Q^T m-tiles (4 x [128, TB]) ----------------------
                for m in range(4):
                    pq = psMM.tile([P, TB], F32, tag="mm512", name="pq")
                    for c in range(NCT):
                        nc.tensor.matmul(
                            pq[:], wq_sb[:, m, c, :], xc(c),
                            start=(c == 0), stop=(c == NCT - 1),
                        )
                    rope_q(q_sb[:, m, tsl], pq, tb)

                # ---- V: project V^T then PE-transpose to [t, d] -------
                pvt = psMM.tile([P, TB], F32, tag="mm512", name="pvt")
                for c in range(NCT):
                    nc.tensor.matmul(
                        pvt[:], wv_sb[:, c, :], xc(c),
                        start=(c == 0), stop=(c == NCT - 1),
                    )
                vt_sb = vtp.tile([P, TB], MMDT, tag="vt")
                nc.vector.tensor_copy(vt_sb[:], pvt[:])
                for s in range(TB // P):
                    kt = tb * (TB // P) + s
                    ptr = psMM.tile([P, P], MMDT, tag="mm512", name="ptr")
                    nc.tensor.transpose(
                        ptr[:], vt_sb[:, s * P : (s + 1) * P], id_sb[:]
                    )
                    for hh in range(2):
                        nc.vector.tensor_copy(
                            v_sb[:, kt, hh, 0:D],
                            ptr[:, hh * D : (hh + 1) * D],
                        )

            def attn_block(jq):
                qb = jq * QB
                nkt = 4 * jq + 4
                for pr in range(4):  # head-pair tiles (local heads pr, pr+4)
                    pv = [
                        psPV.tile([D + 1, QB], F32, tag="pv", name=f"pv{hh}")
                        for hh in range(2)
                    ]
                    for kt in range(nkt):
                        j = kt - 4 * jq  # >= 0: diagonal-crossing tile
                        w = QB - P * j if j >= 0 else QB
                        qoff = qb + P * j if j >= 0 else qb
                        ksl = slice(kt * P, (kt + 1) * P)
                        st = psST.tile([P, 2, QB], F32, tag="st")
                        for hh in range(2):
                            nc.tensor.matmul(
                                st[:, hh, 0:w],
                                k_sb[hh][:, ksl],
                                q_sb[:, pr, qoff : qoff + w],
                                start=True,
                                stop=True,
                            )
                        ptile = ptp.tile([P, 2, QB], MMDT, tag="pt")
                        nc.scalar.activation(
                            ptile[:, :, 0:w],
                            st[:, :, 0:w],
                            mybir.ActivationFunctionType.Exp,
                            scale=EXP_SCALE,
                        )
                        if j >= 0:
                            nc.vector.tensor_mul(
                                ptile[:, :, 0:P], ptile[:, :, 0:P], tri_sb[:]
                            )
                        for hh in range(2):
                            nc.tensor.matmul(
                                pv[hh][:, qoff - qb :],
                                v_sb[:, kt, hh, :],
                                ptile[:, hh, 0:w],
                                start=(kt == 0),
                                stop=(kt == nkt - 1),
                            )
                    # normalize by the accumulated sum row
                    for hh in range(2):
                        srow = small.tile([1, QB], F32, tag="srow")
                        nc.vector.tensor_copy(srow[:], pv[hh][D : D + 1, :])
                        rec = small.tile([1, QB], F32, tag="rec")
                        nc.vector.reciprocal_approx_fast(rec[:], srow[:])
                        bc = small.tile([D, QB], F32, tag="bc")
                        nc.gpsimd.partition_broadcast(bc[:], rec[:])
                        nc.vector.tensor_mul(
                            y_sb[hh * D : (hh + 1) * D, pr, qb : qb + QB],
                            pv[hh][0:D, :],
                            bc[:],
                        )

            def outproj_block(jq):
                # ---- output projection for rows jq*TB..(jq+1)*TB ------
                for s in range(4):
                    t = jq * 4 + s
                    ob = ostage.tile([P, C], MMDT, tag="ob")
                    for cb in range(4):
                        csl = slice(cb * 512, (cb + 1) * 512)
                        po = psMM.tile([P, 512], F32, tag="mm512", name="po")
                        for jj in range(4):
                            nc.tensor.matmul(
                                po[:],
                                y_sb[:, jj, t * P : (t + 1) * P],
                                wc_sb[:, cb, jj, :],
                                start=(jj == 0),
                                stop=(jj == 3),
                            )
                        nc.vector.tensor_copy(ob[:, csl], po[:])
                    nc.gpsimd.dma_start(out[t * P : (t + 1) * P, :], ob[:])

            # emission order: x is prefetched one t-block ahead, and
            # out-proj(tb) is deferred two attention blocks so its
            # matmuls are saved as PE fill for the later (exp-paced)
            # attention phases instead of being consumed early
            xh = load_x(0)
            proj_block(0, xh)
            xh = load_x(1)
            attn_block(0)
            proj_block(1, xh)
            xh = load_x(2)
            attn_block(1)
            proj_block(2, xh)
            xh = load_x(3)
            attn_block(2)
            proj_block(3, xh)
            outproj_block(0)
            attn_block(3)
            outproj_block(1)
            outproj_block(2)
            outproj_block(3)

    nc.finalize()
    return nc


def _rope_tables(position_ids):
    t = position_ids.reshape(-1).astype(np.float64)  # [T]
    inv_freq = 1.0 / ROPE_THETA ** (np.arange(0, D, 2, dtype=np.float64) / D)
    freqs = np.outer(t, inv_freq)  # [T, D/2]
    cos = np.repeat(np.cos(freqs), 2, axis=1)  # [T, D] interleaved
    sin = np.repeat(np.sin(freqs), 2, axis=1)
    sign = np.where(np.arange(D) < D // 2, -1.0, 1.0)
    cosT = np.tile(cos.T, (2, 1))            # [128, T]
    sinT = np.tile((sin * sign).T, (2, 1))   # [128, T]
    return np.ascontiguousarray(cosT), np.ascontiguousarray(sinT)


def _head_perm(g):
    # row indices into Wq (and columns of Wc) for core head-group g
    rows = []
    for lh in LOCAL_HEADS:
        h = g * QH + lh
        rows.extend(range(h * D, (h + 1) * D))
    return np.asarray(rows)


def _pack_po(wT, m):
    # [C, m] -> [128, NCT, m]: partition-contiguous tiles for DMA
    return np.ascontiguousarray(wT.reshape(NCT, P, m).transpose(1, 0, 2))


def make_in_maps(x, Wq, Wk, Wv, Wc, position_ids):
    x = np.asarray(x, dtype=np.float32)
    Wq = np.asarray(Wq, dtype=np.float32)
    Wk = np.asarray(Wk, dtype=np.float32)
    Wv = np.asarray(Wv, dtype=np.float32)
    Wc = np.asarray(Wc, dtype=np.float32)
    cosT, sinT = _rope_tables(np.asarray(position_ids))
    # [128, 2, 128] causal mask (q >= k), duplicated for the head pair
    tri = np.broadcast_to(
        np.triu(np.ones((P, P), dtype=np.float32))[:, None, :], (P, 2, P)
    ).copy()
    in_maps = []
    for core in range(8):
        b, g = divmod(core, 4)
        perm = _head_perm(g)
        kv = slice(2 * g * D, (2 * g + 2) * D)
        # x^T [C, T] -> [128, NTB, NCT, TB] partition-contiguous tiles
        xT = x[b].T.reshape(NCT, P, NTB, TB)
        xPk = np.ascontiguousarray(xT.transpose(1, 2, 0, 3))
        # Wq^T [C, 512] -> [128, 4(m), NCT, 128]
        wqT4 = Wq[perm].T.reshape(NCT, P, 4, P)
        wqPk = np.ascontiguousarray(wqT4.transpose(1, 2, 0, 3))
        # Wc^T [512, C] -> [128, 4(cb), 4(jj), 512]
        wcT4 = Wc[:, perm].T.reshape(4, P, 4, 512)
        wcPk = np.ascontiguousarray(wcT4.transpose(1, 2, 0, 3))
        in_maps.append(
            {
                "xP": xPk.astype(NPDT),
                "wqP": wqPk.astype(NPDT),
                "wkP": _pack_po(Wk[kv].T, 2 * D).astype(NPDT),
                "wvP": _pack_po(Wv[kv].T, 2 * D).astype(NPDT),
                "wcP": wcPk.astype(NPDT),
                "cosT": cosT.astype(NPDT),
                "sinT": sinT.astype(NPDT),
                "tri": tri.astype(NPDT),
                "ident": np.eye(P, dtype=np.float32).astype(NPDT),
            }
        )
    return in_maps


_NC = None


def get_nc():
    global _NC
    if _NC is None:
        _NC = build_bass()
    return _NC


def run_cores(in_maps, core_ids, **kw):
    return run_bass_kernel_spmd(get_nc(), in_maps, core_ids=core_ids, **kw)


def kernel(x, Wq, Wk, Wv, Wc, position_ids, _trace=False, _res_out=None):
    in_maps = make_in_maps(x, Wq, Wk, Wv, Wc, position_ids)
    res = run_cores(in_maps, list(range(8)), trace=_trace)
    if _res_out is not None:
        _res_out.append(res)
    outs = [res.results[i]["out"].astype(np.float32) for i in range(8)]
    y = np.stack(
        [
            outs[0] + outs[1] + outs[2] + outs[3],
            outs[4] + outs[5] + outs[6] + outs[7],
        ]
    )
    return y.astype(np.float32)


# revision 32
# speedup vs baseline: 1.0150x; 1.0150x over previous
"""GQA attention block (B=2, T=2048, C=2048, H=32, Hkv=8, D=64, RoPE, causal)
on 8 TRN2 NeuronCores.

Sharding: core = b*4 + g  (b = batch 0..1, g = head-group 0..3).
Each core computes 8 Q heads / 2 KV heads of one batch element:
  QKV projections -> RoPE -> causal softmax(QK^T/sqrt(D)) V -> partial
  output projection against its 512 columns of Wc.  Host sums the 4
  head-group partials per batch (partials are written in bf16).

Pipeline structure (per core): one fused loop over the 4 sequence blocks.
Iteration tb projects Q/K/V for t-block tb, then runs attention for
q-block tb (which only needs K/V up to block tb), then the output
projection for those rows.  Projection matmuls (PE-heavy) overlap the
previous block's attention (ScalarE-exp-heavy) in the Tile schedule.

Attention computes S^T = K Q^T tiles (k on partitions) so the exp'd
tiles feed the PV matmul with no transposes; a ones-column appended to V
accumulates the softmax denominator in the same matmul; causal masking
skips fully-masked tiles, narrows diagonal-crossing streams, and applies
a 128x128 triangle mask (on GpSimd) to the diagonal block.

Every matmul runs in the full 128x128 PE mode to avoid tile-mode-switch
drains: the per-head K tiles are stored zero-padded to 128 contraction
rows (the other head's rows are 0).  All DRAM inputs are host-packed so
each SBUF tile is a contiguous per-partition read, and output partials
are written in bf16.

Matmul operands are bf16 (KERNEL_MM_DTYPE=f32r selects float32r:
slower, lower error); PSUM accumulation is always fp32.
"""

import os

import ml_dtypes
import numpy as np

import concourse.bacc as bacc
import concourse.mybir as mybir
from concourse.tile import TileContext
from concourse.bass_utils import run_bass_kernel_spmd

B, T, C = 2, 2048, 2048
H, HKV, D = 32, 8, 64
ROPE_THETA = 10000.0

P = 128
NCT = C // P          # 16 contraction subtiles
TB = 512              # t-block width
NTB = T // TB         # 4
QB = 512              # q-block width in attention
KT = T // P           # 16 k-tiles
QH = H // 4           # 8 local q heads per core
LOCAL_HEADS = [0, 4, 1, 5, 2, 6, 3, 7]  # pair (p, p+4) shares a 128-row tile

F32 = mybir.dt.float32
F32R = mybir.dt.float32r
BF16 = mybir.dt.bfloat16

MM_MODE = os.environ.get("KERNEL_MM_DTYPE", "bf16")
MMDT = BF16 if MM_MODE == "bf16" else F32R
NPDT = ml_dtypes.bfloat16 if MM_MODE == "bf16" else np.float32

EXP_SCALE = float(1.0 / np.sqrt(D))


def build_bass():
    nc = bacc.Bacc("TRN2", target_bir_lowering=False, debug=False, num_devices=8)

    # All inputs are pre-packed on the host so every SBUF tile is a
    # contiguous per-partition DRAM read (128 x >=4KB DMA descriptors).
    xP = nc.dram_tensor("xP", [P, NTB, NCT, TB], MMDT, kind="ExternalInput")
    wqP = nc.dram_tensor("wqP", [P, 4, NCT, P], MMDT, kind="ExternalInput")
    wkP = nc.dram_tensor("wkP", [P, NCT, 2 * D], MMDT, kind="ExternalInput")
    wvP = nc.dram_tensor("wvP", [P, NCT, 2 * D], MMDT, kind="ExternalInput")
    wcP = nc.dram_tensor("wcP", [P, 4, 4, 512], MMDT, kind="ExternalInput")
    cosT = nc.dram_tensor("cosT", [P, T], MMDT, kind="ExternalInput")
    sinT = nc.dram_tensor("sinT", [P, T], MMDT, kind="ExternalInput")
    tri = nc.dram_tensor("tri", [P, 2, P], MMDT, kind="ExternalInput")
    ident = nc.dram_tensor("ident", [P, P], MMDT, kind="ExternalInput")
    out = nc.dram_tensor("out", [T, C], MMDT, kind="ExternalOutput")

    with TileContext(nc) as tc:
        with (
            tc.tile_pool(name="persist", bufs=1) as persist,
            tc.tile_pool(name="small", bufs=4) as small,
            tc.tile_pool(name="xs", bufs=8) as xs,
            tc.tile_pool(name="rot", bufs=3) as rotp,
            tc.tile_pool(name="vt", bufs=2) as vtp,
            tc.tile_pool(name="pt", bufs=6) as ptp,
            tc.tile_pool(name="ostage", bufs=2) as ostage,
            tc.tile_pool(name="psMM", bufs=2, space="PSUM") as psMM,
            tc.tile_pool(name="psST", bufs=2, space="PSUM") as psST,
            tc.tile_pool(name="psPV", bufs=2, space="PSUM") as psPV,
        ):
            # ---- persistent SBUF tensors ------------------------------
            q_sb = persist.tile([P, 4, T], MMDT)          # Q^T (rope'd)
            # per-head K^T, zero-padded to 128 contraction rows so the
            # QK matmul stays in 128x128 PE mode (no tile-mode switch)
            k_sb = [
                persist.tile([P, T], MMDT, tag=f"k{h}", name=f"k{h}")
                for h in range(2)
            ]
            v_sb = persist.tile([P, KT, 2, D + 1], MMDT)  # V + ones col
            y_sb = persist.tile([P, 4, T], MMDT)          # attn out^T
            tri_sb = persist.tile([P, 2, P], MMDT)
            id_sb = persist.tile([P, P], MMDT)
            cos_sb = persist.tile([P, T], MMDT)
            sin_sb = persist.tile([P, T], MMDT)
            wk_sb = persist.tile([P, NCT, 2 * D], MMDT, tag="wk")
            wv_sb = persist.tile([P, NCT, 2 * D], MMDT, tag="wv")
            wq_sb = persist.tile([P, 4, NCT, P], MMDT, tag="wq")
            wc_sb = persist.tile([P, 4, 4, 512], MMDT, tag="wc")

            nc.sync.dma_start(wk_sb[:], wkP[:])
            # zero the dead half of each padded K tile (once)
            nc.vector.memset(k_sb[0][D:P, :], 0.0)
            nc.vector.memset(k_sb[1][0:D, :], 0.0)
            nc.vector.memset(v_sb[:, :, :, D], 1.0)  # softmax-denominator col

            def rope_q(dst, psum, tb):
                # dst/psum: [128, TB]; q_rope = q*cos + rot(q)*sin'
                tmp = rotp.tile([P, TB], MMDT, tag="rp_t")
                rtmp = rotp.tile([P, TB], MMDT, tag="rp_r")
                nc.scalar.copy(tmp[:], psum[:])
                for olo, ilo in ((0, 32), (32, 0), (64, 96), (96, 64)):
                    nc.sync.dma_start(
                        rtmp[olo : olo + 32, :], tmp[ilo : ilo + 32, :]
                    )
                ts = slice(tb * TB, (tb + 1) * TB)
                nc.vector.tensor_mul(dst, tmp[:], cos_sb[:, ts])
                nc.vector.tensor_mul(rtmp[:], rtmp[:], sin_sb[:, ts])
                nc.vector.tensor_add(dst, dst, rtmp[:])

            def rope_k(psum, tb):
                # like rope_q but writes each head's rows into its padded tile
                tmp = rotp.tile([P, TB], MMDT, tag="rp_t")
                rtmp = rotp.tile([P, TB], MMDT, tag="rp_r")
                nc.scalar.copy(tmp[:], psum[:])
                for olo, ilo in ((0, 32), (32, 0), (64, 96), (96, 64)):
                    nc.sync.dma_start(
                        rtmp[olo : olo + 32, :], tmp[ilo : ilo + 32, :]
                    )
                ts = slice(tb * TB, (tb + 1) * TB)
                for hh in range(2):
                    rs = slice(hh * D, (hh + 1) * D)
                    dst = k_sb[hh][rs, ts]
                    nc.vector.tensor_mul(dst, tmp[rs, :], cos_sb[rs, ts])
                    nc.vector.tensor_mul(rtmp[rs, :], rtmp[rs, :], sin_sb[rs, ts])
                    nc.vector.tensor_add(dst, dst, rtmp[rs, :])

            def load_x(tb):
                xh = []
                for qtr in range(4):
                    xb = xs.tile(
                        [P, NCT // 4, TB], MMDT, tag="xb", name=f"xb{qtr}"
                    )
                    nc.sync.dma_start(
                        xb[:],
                        xP[:, tb, qtr * (NCT // 4) : (qtr + 1) * (NCT // 4), :],
                    )
                    xh.append(xb)
                return xh

            def proj_block(tb, xh):
                tsl = slice(tb * TB, (tb + 1) * TB)

                def xc(c):
                    return xh[c // (NCT // 4)][:, c % (NCT // 4), :]

                if tb == 0:
                    # weight loads ride the SWDGE queue so they stream in
                    # parallel with the x loads on the sync HWDGE ring;
                    # wq is split per m-tile so Q chains start early
                    for m in range(4):
                        nc.gpsimd.dma_start(wq_sb[:, m], wqP[:, m])
                    nc.sync.dma_start(cos_sb[:], cosT[:])
                    nc.sync.dma_start(sin_sb[:], sinT[:])
                    nc.gpsimd.dma_start(wv_sb[:], wvP[:])
                    nc.sync.dma_start(tri_sb[:], tri[:])
                    nc.sync.dma_start(id_sb[:], ident[:])
                    for cb in range(4):
                        nc.gpsimd.dma_start(wc_sb[:, cb], wcP[:, cb])

                # ---- K^T projection (one [128, TB] tile: 2 kv heads) --
                pk = psMM.tile([P, TB], F32, tag="mm512", name="pk")
                for c in range(NCT):
                    nc.tensor.matmul(
                        pk[:], wk_sb[:, c, :], xc(c),
                        start=(c == 0), stop=(c == NCT - 1),
                    )
                rope_k(pk, tb)

                # ---- Q^T m-tiles (4 x [128, TB]) ----------------------
                for m in range(4):
                    pq = psMM.tile([P, TB], F32, tag="mm512", name="pq")
                    for c in range(NCT):
                        nc.tensor.matmul(
                            pq[:], wq_sb[:, m, c, :], xc(c),
                            start=(c == 0), stop=(c == NCT - 1),
                        )
                    rope_q(q_sb[:, m, tsl], pq, tb)

                # ---- V: project V^T then PE-transpose to [t, d] -------
                pvt = psMM.tile([P, TB], F32, tag="mm512", name="pvt")
                for c in range(NCT):
                    nc.tensor.matmul(
                        pvt[:], wv_sb[:, c, :], xc(c),
                        start=(c == 0), stop=(c == NCT - 1),
                    )
                vt_sb = vtp.tile([P, TB], MMDT, tag="vt")
                nc.vector.tensor_copy(vt_sb[:], pvt[:])
                for s in range(TB // P):
                    kt = tb * (TB // P) + s
                    ptr = psMM.tile([P, P], MMDT, tag="mm512", name="ptr")
                    nc.tensor.transpose(
                        ptr[:], vt_sb[:, s * P : (s + 1) * P], id_sb[:]
                    )
                    for hh in range(2):
                        nc.vector.tensor_copy(
                            v_sb[:, kt, hh, 0:D],
                            ptr[:, hh * D : (hh + 1) * D],
                        )

            def attn_block(jq):
                qb = jq * QB
                nkt = 4 * jq + 4
                for pr in range(4):  # head-pair tiles (local heads pr, pr+4)
                    pv = [
                        psPV.tile([D + 1, QB], F32, tag="pv", name=f"pv{hh}")
                        for hh in range(2)
                    ]
                    for kt in range(nkt):
                        j = kt - 4 * jq  # >= 0: diagonal-crossing tile
                        w = QB - P * j if j >= 0 else QB
                        qoff = qb + P * j if j >= 0 else qb
                        ksl = slice(kt * P, (kt + 1) * P)
                        st = psST.tile([P, 2, QB], F32, tag="st")
                        for hh in range(2):
                            nc.tensor.matmul(
                                st[:, hh, 0:w],
                                k_sb[hh][:, ksl],
                                q_sb[:, pr, qoff : qoff + w],
                                start=True,
                                stop=True,
                            )
                        ptile = ptp.tile([P, 2, QB], MMDT, tag="pt")
                        nc.scalar.activation(
                            ptile[:, :, 0:w],
                            st[:, :, 0:w],
                            mybir.ActivationFunctionType.Exp,
                            scale=EXP_SCALE,
                        )
                        if j >= 0:
                            nc.vector.tensor_mul(
                                ptile[:, :, 0:P], ptile[:, :, 0:P], tri_sb[:]
                            )
                        for hh in range(2):
                            nc.tensor.matmul(
                                pv[hh][:, qoff - qb :],
                                v_sb[:, kt, hh, :],
                                ptile[:, hh, 0:w],
                                start=(kt == 0),
                                stop=(kt == nkt - 1),
                            )
                    # normalize by the accumulated sum row
                    for hh in range(2):
                        srow = small.tile([1, QB], F32, tag="srow")
                        nc.vector.tensor_copy(srow[:], pv[hh][D : D + 1, :])
                        rec = small.tile([1, QB], F32, tag="rec")
                        nc.vector.reciprocal_approx_fast(rec[:], srow[:])
                        bc = small.tile([D, QB], F32, tag="bc")
                        nc.gpsimd.partition_broadcast(bc[:], rec[:])
                        nc.vector.tensor_mul(
                            y_sb[hh * D : (hh + 1) * D, pr, qb : qb + QB],
                            pv[hh][0:D, :],
                            bc[:],
                        )

            def outproj_block(jq):
                # ---- output projection for rows jq*TB..(jq+1)*TB ------
                for s in range(4):
                    t = jq * 4 + s
                    ob = ostage.tile([P, C], MMDT, tag="ob")
                    for cb in range(4):
                        csl = slice(cb * 512, (cb + 1) * 512)
                        po = psMM.tile([P, 512], F32, tag="mm512", name="po")
                        for jj in range(4):
                            nc.tensor.matmul(
                                po[:],
                                y_sb[:, jj, t * P : (t + 1) * P],
                                wc_sb[:, cb, jj, :],
                                start=(jj == 0),
                                stop=(jj == 3),
                            )
                        nc.vector.tensor_copy(ob[:, csl], po[:])
                    nc.scalar.dma_start(out[t * P : (t + 1) * P, :], ob[:])

            # emission order: out-proj(tb) is deferred until after
            # attn(tb+1) so its matmuls are saved as PE fill for the
            # later (exp-paced) attention phases instead of being
            # consumed early
            proj_block(0, load_x(0))
            attn_block(0)
            for tb in range(1, NTB):
                proj_block(tb, load_x(tb))
                attn_block(tb)
                outproj_block(tb - 1)
            outproj_block(NTB - 1)

    nc.finalize()
    return nc


def _rope_tables(position_ids):
    t = position_ids.reshape(-1).astype(np.float64)  # [T]
    inv_freq = 1.0 / ROPE_THETA ** (np.arange(0, D, 2, dtype=np.float64) / D)
    freqs = np.outer(t, inv_freq)  # [T, D/2]
    cos = np.repeat(np.cos(freqs), 2, axis=1)  # [T, D] interleaved
    sin = np.repeat(np.sin(freqs), 2, axis=1)
    sign = np.where(np.arange(D) < D // 2, -1.0, 1.0)
    cosT = np.tile(cos.T, (2, 1))            # [128, T]
    sinT = np.tile((sin * sign).T, (2, 1))   # [128, T]
    return np.ascontiguousarray(cosT), np.ascontiguousarray(sinT)


def _head_perm(g):
    # row indices into Wq (and columns of Wc) for core head-group g
    rows = []
    for lh in LOCAL_HEADS:
        h = g * QH + lh
        rows.extend(range(h * D, (h + 1) * D))
    return np.asarray(rows)


def _pack_po(wT, m):
    # [C, m] -> [128, NCT, m]: partition-contiguous tiles for DMA
    return np.ascontiguousarray(wT.reshape(NCT, P, m).transpose(1, 0, 2))


def make_in_maps(x, Wq, Wk, Wv, Wc, position_ids):
    x = np.asarray(x, dtype=np.float32)
    Wq = np.asarray(Wq, dtype=np.float32)
    Wk = np.asarray(Wk, dtype=np.float32)
    Wv = np.asarray(Wv, dtype=np.float32)
    Wc = np.asarray(Wc, dtype=np.float32)
    cosT, sinT = _rope_tables(np.asarray(position_ids))
    # [128, 2, 128] causal mask (q >= k), duplicated for the head pair
    tri = np.broadcast_to(
        np.triu(np.ones((P, P), dtype=np.float32))[:, None, :], (P, 2, P)
    ).copy()
    in_maps = []
    for core in range(8):
        b, g = divmod(core, 4)
        perm = _head_perm(g)
        kv = slice(2 * g * D, (2 * g + 2) * D)
        # x^T [C, T] -> [128, NTB, NCT, TB] partition-contiguous tiles
        xT = x[b].T.reshape(NCT, P, NTB, TB)
        xPk = np.ascontiguousarray(xT.transpose(1, 2, 0, 3))
        # Wq^T [C, 512] -> [128, 4(m), NCT, 128]
        wqT4 = Wq[perm].T.reshape(NCT, P, 4, P)
        wqPk = np.ascontiguousarray(wqT4.transpose(1, 2, 0, 3))
        # Wc^T [512, C] -> [128, 4(cb), 4(jj), 512]
        wcT4 = Wc[:, perm].T.reshape(4, P, 4, 512)
        wcPk = np.ascontiguousarray(wcT4.transpose(1, 2, 0, 3))
        in_maps.append(
            {
                "xP": xPk.astype(NPDT),
                "wqP": wqPk.astype(NPDT),
                "wkP": _pack_po(Wk[kv].T, 2 * D).astype(NPDT),
                "wvP": _pack_po(Wv[kv].T, 2 * D).astype(NPDT),
                "wcP": wcPk.astype(NPDT),
                "cosT": cosT.astype(NPDT),
                "sinT": sinT.astype(NPDT),
                "tri": tri.astype(NPDT),
                "ident": np.eye(P, dtype=np.float32).astype(NPDT),
            }
        )
    return in_maps


_NC = None


def get_nc():
    global _NC
    if _NC is None:
        _NC = build_bass()
    return _NC


def run_cores(in_maps, core_ids, **kw):
    return run_bass_kernel_spmd(get_nc(), in_maps, core_ids=core_ids, **kw)


def kernel(x, Wq, Wk, Wv, Wc, position_ids, _trace=False, _res_out=None):
    in_maps = make_in_maps(x, Wq, Wk, Wv, Wc, position_ids)
    res = run_cores(in_maps, list(range(8)), trace=_trace)
    if _res_out is not None:
        _res_out.append(res)
    outs = [res.results[i]["out"].astype(np.float32) for i in range(8)]
    y = np.stack(
        [
            outs[0] + outs[1] + outs[2] + outs[3],
            outs[4] + outs[5] + outs[6] + outs[7],
        ]
    )
    return y.astype(np.float32)
